# revision 12
# baseline (speedup 1.0000x reference)
import sys

if "/opt/trn_rl_repo" not in sys.path:
    sys.path.insert(0, "/opt/trn_rl_repo")

import ml_dtypes
import numpy as np

import concourse.bass as bass
import concourse.tile as tile
from concourse import bacc
from concourse import mybir
from concourse.bass_utils import run_bass_kernel_spmd

F32 = mybir.dt.float32
BF16 = mybir.dt.bfloat16
U8 = mybir.dt.uint8
ALU = mybir.AluOpType
ACTF = mybir.ActivationFunctionType

P = 128
TEMPERATURE = 0.6
EPS_NOISE = 1e-4
NCORES = 8

# Full-size layout: each core gets <= 2,500,015 contiguous elements (shards
# snapped to group boundaries), padded to S = P*W.
W_FULL = 19584
LOOK = 64  # > max run length (46)
F_FULL = 1983  # (F + LOOK + 1) * 4B = 8192B = 4 PSUM banks per buffer


def _chunks(W, F):
    out = []
    c = 0
    while c < W:
        out.append((c, min(F, W - c)))
        c += F
    return out


def rev(ap):
    """Reverse an AP along its last (free) axis."""
    a = ap
    pat = [list(p) for p in a.ap]
    n = pat[-1][1]
    assert pat[-1][0] == 1
    pat[-1][0] = -1
    return bass.AP(a.tensor, a.offset + (n - 1), pat)


def build(W, F, look=LOOK):
    """Builds the Bass program for one core's [P, W] shard.

    Per-element math (mask-mult segmented scans; segments never longer than
    `look`, so reverse scans only need a `look`-wide lookahead window):
        B    = ln(-ln u)
        t3   = logits - B                     (gpsimd)
        e    = exp(t3 / T)                    (scalar)
        pref = fwd scan (mb,  e;    *, +)     (vector, mask in PSUM)
        d    = rev scan (mbx, pref; *, max)   == segment total of e
        soft = exp((t3 - T*ln d) / T)         (scalar + gpsimd)
        sn   = soft + EPS * u_eps             (gpsimd)
        pmax = fwd scan (mb,  sn;   *, max)
        m    = rev scan (mbx, pmax; *, max)   == segment max of sn
        hot  = (sn == m)                      (gpsimd)

    Three-stage software pipeline per python iteration `it`:
        produce(it):   DMA + Gumbel transform + e        for chunk it
        sums(it-1):    sum scans + soft + sn             for chunk it-1
        maxhot(it-2):  max scans + hot + output DMA      for chunk it-2
    so no engine queue head-blocks on a cross-engine dependency.

    Cross-partition (row-boundary) segments are fixed up at the end from
    stashed head/tail windows, with all masks derived from mb.
    """
    nc = bacc.Bacc("TRN2", target_bir_lowering=False, debug=False)
    u_d = nc.dram_tensor("u", [P, W], F32, kind="ExternalInput")
    l_d = nc.dram_tensor("l", [P, W], F32, kind="ExternalInput")
    ue_d = nc.dram_tensor("ue", [P, W], F32, kind="ExternalInput")
    mb_d = nc.dram_tensor("mb", [P, W + 1], BF16, kind="ExternalInput")
    id_d = nc.dram_tensor("ident", [P, P], BF16, kind="ExternalInput")
    soft_d = nc.dram_tensor("soft", [P, W], F32, kind="ExternalOutput")
    hot_d = nc.dram_tensor("hot", [P, W], U8, kind="ExternalOutput")

    chunks = _chunks(W, F)
    nch = len(chunks)
    inv_t = 1.0 / TEMPERATURE
    FL = F + look

    with tile.TileContext(nc) as tc:
        with (
            tc.tile_pool(name="main", bufs=2) as pool,
            tc.tile_pool(name="lw3", bufs=3) as lwpool,
            tc.tile_pool(name="msk", bufs=2, space="PSUM") as pmask,
            tc.tile_pool(name="fix", bufs=1) as fx,
        ):
            # persistent stash tiles for the cross-partition fixup
            mbH = fx.tile([P, look], F32, tag="mbH")
            mbTx = fx.tile([P, look], F32, tag="mbTx")
            eH = fx.tile([P, look], F32, tag="eH")
            ueH = fx.tile([P, look], F32, tag="ueH")
            snH = fx.tile([P, look], F32, tag="snH")
            softH = fx.tile([P, look], F32, tag="softH")
            smH = fx.tile([P, look], F32, tag="smH")
            eT = fx.tile([P, look], F32, tag="eT")
            ueT = fx.tile([P, look], F32, tag="ueT")
            snT = fx.tile([P, look], F32, tag="snT")
            softT = fx.tile([P, look], F32, tag="softT")
            smT = fx.tile([P, look], F32, tag="smT")
            cont = fx.tile([P, 1], F32, tag="cont")

            # identity weights for the PE mask copy into PSUM
            ident = fx.tile([P, P], BF16, tag="ident")
            nc.sync.dma_start(ident[:], id_d.ap()[:])

            # per-chunk live state, keyed by chunk index
            live = {}
            carry = {"pref": None, "pmax": None}

            def produce(ci):
                c0, F_c = chunks[ci]
                first = ci == 0
                last = ci == nch - 1
                Fw = F_c + look if not last else F_c

                uw = pool.tile([P, FL], F32, name="uw", tag="uw")[:, 0:Fw]
                lw = lwpool.tile([P, FL], F32, name="lw", tag="lw")[:, 0:Fw]
                uew = pool.tile([P, FL], F32, name="uew", tag="uew")[:, 0:Fw]
                nc.sync.dma_start(uw[:], u_d.ap()[:, c0 : c0 + Fw])
                nc.sync.dma_start(lw[:], l_d.ap()[:, c0 : c0 + Fw])
                nc.sync.dma_start(uew[:], ue_d.ap()[:, c0 : c0 + Fw])

                # B = ln(-ln(u))  in place over uw
                nc.scalar.activation(uw[:], uw[:], ACTF.Ln)
                nc.scalar.activation(uw[:], uw[:], ACTF.Ln, scale=-1.0)
                # t3 = logits - B   (in lw)
                nc.gpsimd.tensor_tensor(out=lw[:], in0=lw[:], in1=uw[:], op=ALU.subtract)
                e = pool.tile([P, FL], F32, name="e", tag="e")[:, 0:Fw]
                nc.scalar.activation(e[:], lw[:], ACTF.Exp, scale=inv_t)
                live[ci] = {"lw": lw, "uew": uew, "e": e, "Fw": Fw}

            def sums_cast(ci):
                # mask lands in SBUF as bf16, then the idle tensor engine
                # copies it into PSUM as f32 via an identity matmul (exact
                # for 0/1 masks).  The DVE scans then use only their
                # dedicated SBUF port pair + the PSUM port, leaving the
                # shared pair entirely to gpsimd.
                c0, _ = chunks[ci]
                st = live[ci]
                Fw = st["Fw"]
                mbs = pool.tile([P, FL + 1], BF16, name="mbs", tag="mbs")
                nc.sync.dma_start(mbs[:, 0 : Fw + 1], mb_d.ap()[:, c0 : c0 + Fw + 1])
                mbf = pmask.tile([P, FL + 1], F32, name="mbf", tag="mbf")
                for j in range(0, Fw + 1, 512):
                    n = min(512, Fw + 1 - j)
                    nc.tensor.matmul(mbf[:, j : j + n], ident[:], mbs[:, j : j + n])
                st["mbf"] = mbf
                if ci == 0:
                    nc.vector.tensor_copy(out=mbH[:], in_=mbf[:, 0:look])
                    nc.vector.tensor_copy(out=cont[:], in_=mbf[:, 0:1])
                if ci == nch - 1:
                    F_c = chunks[ci][1]
                    # mbTx[k] = mb[W-64+k+1] for k<63 ; mbTx[63] forced to 1
                    nc.vector.tensor_copy(
                        out=mbTx[:, 0 : look - 1],
                        in_=mbf[:, F_c - look + 1 : F_c],
                    )
                    nc.vector.memset(mbTx[:, look - 1 : look], 1)

            def sums(ci):
                c0, F_c = chunks[ci]
                first = ci == 0
                last = ci == nch - 1
                st = live[ci]
                Fw = st["Fw"]
                e, lw, uew = st["e"], st["lw"], st["uew"]
                mbf = st["mbf"]
                mb = mbf[:, 0:Fw]
                mbx = mbf[:, 1 : Fw + 1]

                pref = pool.tile([P, FL], F32, name="pref", tag="pref")[:, 0:Fw]
                init = 0.0 if first else carry["pref"]
                nc.vector.tensor_tensor_scan(
                    out=pref, data0=mb, data1=e, initial=init,
                    op0=ALU.mult, op1=ALU.add,
                )
                dbuf = pool.tile([P, FL], F32, name="dbuf", tag="dbuf")[:, 0:Fw]
                nc.vector.tensor_tensor_scan(
                    out=rev(dbuf), data0=rev(mbx), data1=rev(pref), initial=0.0,
                    op0=ALU.mult, op1=ALU.max,
                )
                carry["pref"] = pref[:, F_c - 1 : F_c]

                # rd = 1/d via exp(-ln d), in place over dbuf; then
                # soft = e * rd and sn = soft + ue' (ue' = EPS * u_eps,
                # prescaled host-side) -- plain tensor_tensor, Pool-legal
                nc.scalar.activation(dbuf, dbuf, ACTF.Ln)
                nc.scalar.activation(dbuf, dbuf, ACTF.Exp, scale=-1.0)
                soft = pool.tile([P, FL], F32, name="soft", tag="soft")[:, 0:Fw]
                nc.gpsimd.tensor_tensor(out=soft, in0=e, in1=dbuf, op=ALU.mult)
                # sn into lw; t3 dead
                nc.gpsimd.tensor_tensor(out=lw[:], in0=soft, in1=uew[:], op=ALU.add)

                if first:
                    nc.scalar.copy(eH[:], e[:, 0:look])
                    nc.scalar.copy(ueH[:], uew[:, 0:look])
                    nc.scalar.copy(softH[:], soft[:, 0:look])
                    nc.scalar.copy(snH[:], lw[:, 0:look])
                if last:
                    nc.scalar.copy(eT[:], e[:, F_c - look : F_c])
                    nc.scalar.copy(ueT[:], uew[:, F_c - look : F_c])
                    nc.scalar.copy(softT[:], soft[:, F_c - look : F_c])
                    nc.scalar.copy(snT[:], lw[:, F_c - look : F_c])
                st["soft"] = soft

            def maxhot(ci):
                c0, F_c = chunks[ci]
                first = ci == 0
                last = ci == nch - 1
                st = live.pop(ci)
                Fw = st["Fw"]
                sn, soft, mbf = st["lw"], st["soft"], st["mbf"]
                mb = mbf[:, 0:Fw]
                mbx = mbf[:, 1 : Fw + 1]

                pmax = pool.tile([P, FL], F32, name="pmax", tag="pmax")[:, 0:Fw]
                initm = 0.0 if first else carry["pmax"]
                nc.vector.tensor_tensor_scan(
                    out=pmax, data0=mb, data1=sn, initial=initm,
                    op0=ALU.mult, op1=ALU.max,
                )
                m = pool.tile([P, FL], F32, name="m", tag="m")[:, 0:Fw]
                nc.vector.tensor_tensor_scan(
                    out=rev(m), data0=rev(mbx), data1=rev(pmax), initial=0.0,
                    op0=ALU.mult, op1=ALU.max,
                )
                carry["pmax"] = pmax[:, F_c - 1 : F_c]

                # diff = sn - m (exact zero iff equal), then a one-port
                # tensor_scalar compare on DVE: no shared-SBUF-port traffic
                nc.gpsimd.tensor_tensor(out=sn, in0=sn, in1=m, op=ALU.subtract)
                hot = pool.tile([P, FL], U8, name="hot", tag="hot")[:, 0:Fw]
                nc.vector.tensor_scalar(
                    out=hot, in0=sn, scalar1=0.0, scalar2=None, op0=ALU.is_equal
                )

                if first:
                    nc.scalar.copy(smH[:], m[:, 0:look])
                if last:
                    nc.scalar.copy(smT[:], m[:, F_c - look : F_c])

                a = look if first else 0
                b = F_c - look if last else F_c
                nc.sync.dma_start(soft_d.ap()[:, c0 + a : c0 + b], soft[:, a:b])
                nc.sync.dma_start(hot_d.ap()[:, c0 + a : c0 + b], hot[:, a:b])

            for it in range(nch + 2):
                if 0 <= it - 1 < nch:
                    sums_cast(it - 1)
                if it < nch:
                    produce(it)
                if 0 <= it - 1 < nch:
                    sums(it - 1)
                if 0 <= it - 2 < nch:
                    maxhot(it - 2)

            # ---------------- cross-partition fixup ----------------
            ones = fx.tile([P, look], F32, tag="ones")
            lm = fx.tile([P, look], F32, tag="lm")
            fm = fx.tile([P, look], F32, tag="fm")
            mbHf = fx.tile([P, look], F32, tag="mbHf")
            mbTf = fx.tile([P, look], F32, tag="mbTf")
            TS = fx.tile([P, 1], F32, tag="TS")
            HS = fx.tile([P, 1], F32, tag="HS")
            TS_sh = fx.tile([P, 1], F32, tag="TS_sh")
            TB = fx.tile([P, 1], F32, tag="TB")
            TBd = fx.tile([P, 1], F32, tag="TBd")
            rB = fx.tile([P, 1], F32, tag="rB")
            rT = fx.tile([P, 1], F32, tag="rT")
            contU = fx.tile([P, 1], F32, tag="contU")
            tmpH = fx.tile([P, look], F32, tag="tmpH")
            tmpT = fx.tile([P, look], F32, tag="tmpT")
            affH = fx.tile([P, look], F32, tag="affH")
            affT = fx.tile([P, look], F32, tag="affT")
            softHn = fx.tile([P, look], F32, tag="softHn")
            softTn = fx.tile([P, look], F32, tag="softTn")
            snHn = fx.tile([P, look], F32, tag="snHn")
            snTn = fx.tile([P, look], F32, tag="snTn")
            mH = fx.tile([P, 1], F32, tag="mH")
            mT = fx.tile([P, 1], F32, tag="mT")
            mTd = fx.tile([P, 1], F32, tag="mTd")
            mHu = fx.tile([P, 1], F32, tag="mHu")
            rmH = fx.tile([P, 1], F32, tag="rmH")
            rmT = fx.tile([P, 1], F32, tag="rmT")
            e1 = fx.tile([P, look], F32, tag="e1")
            e0 = fx.tile([P, look], F32, tag="e0")
            hfH = fx.tile([P, look], F32, tag="hfH")
            hfT = fx.tile([P, look], F32, tag="hfT")
            affHu = fx.tile([P, look], U8, tag="affHu")
            affTu = fx.tile([P, look], U8, tag="affTu")

            # fm[k] = all(mb[1..k]) within head window; fm[0] = 1
            nc.vector.memset(ones[:], 1.0)
            nc.vector.tensor_copy(out=mbHf[:], in_=mbH[:])
            nc.vector.memset(mbHf[:, 0:1], 1.0)
            nc.vector.tensor_tensor_scan(
                out=fm[:], data0=mbHf[:], data1=ones[:], initial=1.0,
                op0=ALU.mult, op1=ALU.mult,
            )
            # lm[k] = all(mbT[k+1..63]) within tail window; lm[63] = 1
            nc.vector.tensor_copy(out=mbTf[:], in_=mbTx[:])
            nc.vector.tensor_tensor_scan(
                out=rev(lm[:]), data0=rev(mbTf[:]), data1=rev(ones[:]), initial=1.0,
                op0=ALU.mult, op1=ALU.mult,
            )
            # cont[0] = 0: first row of a shard always starts a new segment
            nc.vector.memset(cont[0:1, :], 0)

            # tail/head partial sums of e over the boundary run
            nc.vector.tensor_tensor(out=tmpT[:], in0=eT[:], in1=lm[:], op=ALU.mult)
            nc.vector.tensor_reduce(
                out=TS[:], in_=tmpT[:], axis=mybir.AxisListType.X, op=ALU.add
            )
            nc.vector.tensor_tensor(out=tmpH[:], in0=eH[:], in1=fm[:], op=ALU.mult)
            nc.vector.tensor_reduce(
                out=HS[:], in_=tmpH[:], axis=mybir.AxisListType.X, op=ALU.add
            )
            nc.vector.memset(TS_sh[:], 1.0)
            nc.sync.dma_start(TS_sh[1:P, :], TS[0 : P - 1, :])
            nc.vector.tensor_tensor(out=TB[:], in0=TS_sh[:], in1=HS[:], op=ALU.add)
            nc.vector.tensor_scalar(
                out=TB[:], in0=TB[:], scalar1=1e-30, scalar2=None, op0=ALU.max
            )
            nc.vector.memset(TBd[:], 1.0)
            nc.sync.dma_start(TBd[0 : P - 1, :], TB[1:P, :])
            nc.vector.memset(contU[:], 0.0)
            nc.sync.dma_start(contU[0 : P - 1, :], cont[1:P, :])
            nc.vector.reciprocal(rB[:], TB[:])
            nc.vector.reciprocal(rT[:], TBd[:])

            # corrected values, head side
            nc.vector.tensor_scalar(
                out=affH[:], in0=fm[:], scalar1=cont[:], scalar2=None, op0=ALU.mult
            )
            nc.vector.tensor_scalar(
                out=softHn[:], in0=eH[:], scalar1=rB[:], scalar2=None, op0=ALU.mult
            )
            nc.vector.tensor_tensor(out=snHn[:], in0=softHn[:], in1=ueH[:], op=ALU.add)
            # corrected values, tail side
            nc.vector.tensor_scalar(
                out=affT[:], in0=lm[:], scalar1=contU[:], scalar2=None, op0=ALU.mult
            )
            nc.vector.tensor_scalar(
                out=softTn[:], in0=eT[:], scalar1=rT[:], scalar2=None, op0=ALU.mult
            )
            nc.vector.tensor_tensor(out=snTn[:], in0=softTn[:], in1=ueT[:], op=ALU.add)
            # per-side run maxima over affected elements
            nc.vector.tensor_tensor(out=tmpH[:], in0=snHn[:], in1=affH[:], op=ALU.mult)
            nc.vector.tensor_reduce(
                out=mH[:], in_=tmpH[:], axis=mybir.AxisListType.X, op=ALU.max
            )
            nc.vector.tensor_tensor(out=tmpT[:], in0=snTn[:], in1=affT[:], op=ALU.mult)
            nc.vector.tensor_reduce(
                out=mT[:], in_=tmpT[:], axis=mybir.AxisListType.X, op=ALU.max
            )
            nc.vector.memset(mTd[:], 0.0)
            nc.sync.dma_start(mTd[1:P, :], mT[0 : P - 1, :])
            nc.vector.memset(mHu[:], 0.0)
            nc.sync.dma_start(mHu[0 : P - 1, :], mH[1:P, :])
            nc.vector.tensor_tensor(out=rmH[:], in0=mTd[:], in1=mH[:], op=ALU.max)
            nc.vector.tensor_tensor(out=rmT[:], in0=mT[:], in1=mHu[:], op=ALU.max)

            # integer masks for select
            nc.vector.tensor_copy(out=affHu[:], in_=affH[:])
            nc.vector.tensor_copy(out=affTu[:], in_=affT[:])

            # merged soft / hot, head side
            nc.vector.select(softH[:], affHu[:], softHn[:], softH[:])
            nc.vector.tensor_scalar(
                out=e1[:], in0=snHn[:], scalar1=rmH[:], scalar2=None, op0=ALU.is_equal
            )
            nc.vector.tensor_tensor(out=e0[:], in0=snH[:], in1=smH[:], op=ALU.is_equal)
            nc.vector.select(hfH[:], affHu[:], e1[:], e0[:])
            hu8H = fx.tile([P, look], U8, tag="hu8H")
            nc.vector.tensor_copy(out=hu8H[:], in_=hfH[:])
            # merged, tail side
            nc.vector.select(softT[:], affTu[:], softTn[:], softT[:])
            nc.vector.tensor_scalar(
                out=e1[:], in0=snTn[:], scalar1=rmT[:], scalar2=None, op0=ALU.is_equal
            )
            nc.vector.tensor_tensor(out=e0[:], in0=snT[:], in1=smT[:], op=ALU.is_equal)
            nc.vector.select(hfT[:], affTu[:], e1[:], e0[:])
            hu8T = fx.tile([P, look], U8, tag="hu8T")
            nc.vector.tensor_copy(out=hu8T[:], in_=hfT[:])

            nc.sync.dma_start(soft_d.ap()[:, 0:look], softH[:])
            nc.sync.dma_start(hot_d.ap()[:, 0:look], hu8H[:])
            nc.sync.dma_start(soft_d.ap()[:, W - look : W], softT[:])
            nc.sync.dma_start(hot_d.ap()[:, W - look : W], hu8T[:])
    nc.compile()
    return nc


def _prep_shards(logits, logit_groups, u_gumbel, u_eps, W):
    """Split at group boundaries, pad each shard to [P, W] arrays."""
    E = logits.shape[0]
    splits = [0]
    for k in range(1, NCORES):
        t = k * E // NCORES
        splits.append(int(np.searchsorted(logit_groups, logit_groups[t])))
    splits.append(E)

    mb_full = np.zeros(E, dtype=np.float32)
    np.equal(logit_groups[1:], logit_groups[:-1], out=mb_full[1:])

    S = P * W
    in_maps = []
    lens = []
    for k in range(NCORES):
        lo, hi = splits[k], splits[k + 1]
        L = hi - lo
        assert L <= S, (L, S)
        lens.append(L)

        def padded(x, fill, dtype, w=W):
            arr = np.full((P, w), fill, dtype=dtype)
            flat = arr[:, :W].reshape(-1)
            flat[:L] = x
            arr[:, :W] = flat.reshape(P, W)
            return arr

        # row starts keep their true cross-row continuation bit via the
        # [P, W] reshape; padding and the extra column are 0 (every pad
        # element is its own one-element segment). mb[lo] is 0 because
        # shards are snapped to group boundaries.
        mba = padded(mb_full[lo:hi], 0, ml_dtypes.bfloat16, W + 1)
        mba[:, W] = 0
        ua = padded(u_gumbel[lo:hi], 0.5, np.float32)
        la = padded(logits[lo:hi], 0.0, np.float32)
        uea = padded(np.float32(EPS_NOISE) * u_eps[lo:hi], 0.5, np.float32)
        in_maps.append({"u": ua, "l": la, "ue": uea, "mb": mba,
                        "ident": np.eye(P, dtype=ml_dtypes.bfloat16)})
    return in_maps, lens


_CACHE = {}


def kernel(logits, logit_groups, n_groups, u_gumbel, u_eps):
    logits = np.asarray(logits, dtype=np.float32)
    logit_groups = np.asarray(logit_groups, dtype=np.int32)
    u_gumbel = np.asarray(u_gumbel, dtype=np.float32)
    u_eps = np.asarray(u_eps, dtype=np.float32)
    E = logits.shape[0]

    in_maps, lens = _prep_shards(logits, logit_groups, u_gumbel, u_eps, W_FULL)

    if "nc" not in _CACHE:
        _CACHE["nc"] = build(W_FULL, F_FULL)
    nc = _CACHE["nc"]

    res = run_bass_kernel_spmd(nc, in_maps, core_ids=list(range(NCORES)))
    _CACHE["last_res"] = res
    soft = np.empty(E, dtype=np.float32)
    hot = np.empty(E, dtype=np.uint8)
    off = 0
    for k in range(NCORES):
        L = lens[k]
        soft[off : off + L] = res.results[k]["soft"].reshape(-1)[:L]
        hot[off : off + L] = res.results[k]["hot"].reshape(-1)[:L]
        off += L
    assert off == E
    s_hot = hot.astype(np.int32)
    st = hot.astype(np.float32)
    return st, s_hot, soft


# revision 13
# speedup vs baseline: 1.0981x; 1.0981x over previous
import sys

if "/opt/trn_rl_repo" not in sys.path:
    sys.path.insert(0, "/opt/trn_rl_repo")

import ml_dtypes
import numpy as np

import concourse.bass as bass
import concourse.tile as tile
from concourse import bacc
from concourse import mybir
from concourse.bass_utils import run_bass_kernel_spmd

F32 = mybir.dt.float32
BF16 = mybir.dt.bfloat16
U8 = mybir.dt.uint8
ALU = mybir.AluOpType
ACTF = mybir.ActivationFunctionType

P = 128
TEMPERATURE = 0.6
EPS_NOISE = 1e-4
NCORES = 8

# Full-size layout: each core gets <= 2,500,015 contiguous elements (shards
# snapped to group boundaries), padded to S = P*W.
W_FULL = 19584
LOOK = 64  # > max run length (46)
F_FULL = 1983  # (F + LOOK + 1) * 4B = 8192B = 4 PSUM banks per buffer


def _chunks(W, F):
    out = []
    c = 0
    while c < W:
        out.append((c, min(F, W - c)))
        c += F
    return out


def rev(ap):
    """Reverse an AP along its last (free) axis."""
    a = ap
    pat = [list(p) for p in a.ap]
    n = pat[-1][1]
    assert pat[-1][0] == 1
    pat[-1][0] = -1
    return bass.AP(a.tensor, a.offset + (n - 1), pat)


def build(W, F, look=LOOK):
    """Builds the Bass program for one core's [P, W] shard.

    Per-element math (mask-mult segmented scans; segments never longer than
    `look`, so reverse scans only need a `look`-wide lookahead window):
        B    = ln(-ln u)
        t3   = logits - B                     (gpsimd)
        e    = exp(t3 / T)                    (scalar)
        pref = fwd scan (mb,  e;    *, +)     (vector, mask in PSUM)
        d    = rev scan (mbx, pref; *, max)   == segment total of e
        soft = exp((t3 - T*ln d) / T)         (scalar + gpsimd)
        sn   = soft + EPS * u_eps             (gpsimd)
        pmax = fwd scan (mb,  sn;   *, max)
        m    = rev scan (mbx, pmax; *, max)   == segment max of sn
        hot  = (sn == m)                      (gpsimd)

    Three-stage software pipeline per python iteration `it`:
        produce(it):   DMA + Gumbel transform + e        for chunk it
        sums(it-1):    sum scans + soft + sn             for chunk it-1
        maxhot(it-2):  max scans + hot + output DMA      for chunk it-2
    so no engine queue head-blocks on a cross-engine dependency.

    Cross-partition (row-boundary) segments are fixed up at the end from
    stashed head/tail windows, with all masks derived from mb.
    """
    nc = bacc.Bacc("TRN2", target_bir_lowering=False, debug=False)
    u_d = nc.dram_tensor("u", [P, W], F32, kind="ExternalInput")
    l_d = nc.dram_tensor("l", [P, W], F32, kind="ExternalInput")
    ue_d = nc.dram_tensor("ue", [P, W], F32, kind="ExternalInput")
    mb_d = nc.dram_tensor("mb", [P, W + 1], BF16, kind="ExternalInput")
    id_d = nc.dram_tensor("ident", [P, P], BF16, kind="ExternalInput")
    soft_d = nc.dram_tensor("soft", [P, W], F32, kind="ExternalOutput")
    hot_d = nc.dram_tensor("hot", [P, W], U8, kind="ExternalOutput")

    chunks = _chunks(W, F)
    nch = len(chunks)
    inv_t = 1.0 / TEMPERATURE
    FL = F + look

    with tile.TileContext(nc) as tc:
        with (
            tc.tile_pool(name="main", bufs=2) as pool,
            tc.tile_pool(name="lw3", bufs=3) as lwpool,
            tc.tile_pool(name="msk", bufs=2, space="PSUM") as pmask,
            tc.tile_pool(name="fix", bufs=1) as fx,
        ):
            # persistent stash tiles for the cross-partition fixup
            mbH = fx.tile([P, look], F32, tag="mbH")
            mbTx = fx.tile([P, look], F32, tag="mbTx")
            eH = fx.tile([P, look], F32, tag="eH")
            ueH = fx.tile([P, look], F32, tag="ueH")
            snH = fx.tile([P, look], F32, tag="snH")
            softH = fx.tile([P, look], F32, tag="softH")
            smH = fx.tile([P, look], F32, tag="smH")
            eT = fx.tile([P, look], F32, tag="eT")
            ueT = fx.tile([P, look], F32, tag="ueT")
            snT = fx.tile([P, look], F32, tag="snT")
            softT = fx.tile([P, look], F32, tag="softT")
            smT = fx.tile([P, look], F32, tag="smT")
            cont = fx.tile([P, 1], F32, tag="cont")

            # identity weights for the PE mask copy into PSUM
            ident = fx.tile([P, P], BF16, tag="ident")
            nc.sync.dma_start(ident[:], id_d.ap()[:])

            # per-chunk live state, keyed by chunk index
            live = {}
            carry = {"pref": None, "pmax": None}

            def produce(ci):
                c0, F_c = chunks[ci]
                first = ci == 0
                last = ci == nch - 1
                Fw = F_c + look if not last else F_c

                uw = pool.tile([P, FL], F32, name="uw", tag="uw")[:, 0:Fw]
                lw = lwpool.tile([P, FL], F32, name="lw", tag="lw")[:, 0:Fw]
                uew = pool.tile([P, FL], F32, name="uew", tag="uew")[:, 0:Fw]
                nc.sync.dma_start(uw[:], u_d.ap()[:, c0 : c0 + Fw])
                nc.sync.dma_start(lw[:], l_d.ap()[:, c0 : c0 + Fw])
                nc.sync.dma_start(uew[:], ue_d.ap()[:, c0 : c0 + Fw])

                # B = ln(-ln(u))  in place over uw
                nc.scalar.activation(uw[:], uw[:], ACTF.Ln)
                nc.scalar.activation(uw[:], uw[:], ACTF.Ln, scale=-1.0)
                # t3 = logits - B   (in lw)
                nc.gpsimd.tensor_tensor(out=lw[:], in0=lw[:], in1=uw[:], op=ALU.subtract)
                e = pool.tile([P, FL], F32, name="e", tag="e")[:, 0:Fw]
                nc.scalar.activation(e[:], lw[:], ACTF.Exp, scale=inv_t)
                live[ci] = {"lw": lw, "uew": uew, "e": e, "Fw": Fw}

            def sums_cast(ci):
                # mask lands in SBUF as bf16, then the idle tensor engine
                # copies it into PSUM as f32 via an identity matmul (exact
                # for 0/1 masks).  The DVE scans then use only their
                # dedicated SBUF port pair + the PSUM port, leaving the
                # shared pair entirely to gpsimd.
                c0, _ = chunks[ci]
                st = live[ci]
                Fw = st["Fw"]
                mbs = pool.tile([P, FL + 1], BF16, name="mbs", tag="mbs")
                nc.sync.dma_start(mbs[:, 0 : Fw + 1], mb_d.ap()[:, c0 : c0 + Fw + 1])
                mbf = pmask.tile([P, FL + 1], F32, name="mbf", tag="mbf")
                for j in range(0, Fw + 1, 512):
                    n = min(512, Fw + 1 - j)
                    nc.tensor.matmul(mbf[:, j : j + n], ident[:], mbs[:, j : j + n])
                st["mbf"] = mbf
                if ci == 0:
                    nc.vector.tensor_copy(out=mbH[:], in_=mbf[:, 0:look])
                    nc.vector.tensor_copy(out=cont[:], in_=mbf[:, 0:1])
                if ci == nch - 1:
                    F_c = chunks[ci][1]
                    # mbTx[k] = mb[W-64+k+1] for k<63 ; mbTx[63] forced to 1
                    nc.vector.tensor_copy(
                        out=mbTx[:, 0 : look - 1],
                        in_=mbf[:, F_c - look + 1 : F_c],
                    )
                    nc.vector.memset(mbTx[:, look - 1 : look], 1)

            def sums(ci):
                c0, F_c = chunks[ci]
                first = ci == 0
                last = ci == nch - 1
                st = live[ci]
                Fw = st["Fw"]
                e, lw, uew = st["e"], st["lw"], st["uew"]
                mbf = st["mbf"]
                mb = mbf[:, 0:Fw]
                mbx = mbf[:, 1 : Fw + 1]

                pref = pool.tile([P, FL], F32, name="pref", tag="pref")[:, 0:Fw]
                init = 0.0 if first else carry["pref"]
                nc.vector.tensor_tensor_scan(
                    out=pref, data0=mb, data1=e, initial=init,
                    op0=ALU.mult, op1=ALU.add,
                )
                dbuf = pool.tile([P, FL], F32, name="dbuf", tag="dbuf")[:, 0:Fw]
                nc.vector.tensor_tensor_scan(
                    out=rev(dbuf), data0=rev(mbx), data1=rev(pref), initial=0.0,
                    op0=ALU.mult, op1=ALU.max,
                )
                carry["pref"] = pref[:, F_c - 1 : F_c]

                # rd = 1/d via exp(-ln d), in place over dbuf; then
                # soft = e * rd and sn = soft + ue' (ue' = EPS * u_eps,
                # prescaled host-side) -- plain tensor_tensor, Pool-legal
                nc.scalar.activation(dbuf, dbuf, ACTF.Ln)
                nc.scalar.activation(dbuf, dbuf, ACTF.Exp, scale=-1.0)
                soft = pool.tile([P, FL], F32, name="soft", tag="soft")[:, 0:Fw]
                nc.gpsimd.tensor_tensor(out=soft, in0=e, in1=dbuf, op=ALU.mult)
                # sn into lw; t3 dead
                nc.gpsimd.tensor_tensor(out=lw[:], in0=soft, in1=uew[:], op=ALU.add)

                if first:
                    nc.gpsimd.tensor_copy(out=eH[:], in_=e[:, 0:look])
                    nc.gpsimd.tensor_copy(out=ueH[:], in_=uew[:, 0:look])
                    nc.gpsimd.tensor_copy(out=softH[:], in_=soft[:, 0:look])
                    nc.gpsimd.tensor_copy(out=snH[:], in_=lw[:, 0:look])
                if last:
                    nc.gpsimd.tensor_copy(out=eT[:], in_=e[:, F_c - look : F_c])
                    nc.gpsimd.tensor_copy(out=ueT[:], in_=uew[:, F_c - look : F_c])
                    nc.gpsimd.tensor_copy(out=softT[:], in_=soft[:, F_c - look : F_c])
                    nc.gpsimd.tensor_copy(out=snT[:], in_=lw[:, F_c - look : F_c])
                st["soft"] = soft

            def maxhot(ci):
                c0, F_c = chunks[ci]
                first = ci == 0
                last = ci == nch - 1
                st = live.pop(ci)
                Fw = st["Fw"]
                sn, soft, mbf = st["lw"], st["soft"], st["mbf"]
                mb = mbf[:, 0:Fw]
                mbx = mbf[:, 1 : Fw + 1]

                pmax = pool.tile([P, FL], F32, name="pmax", tag="pmax")[:, 0:Fw]
                initm = 0.0 if first else carry["pmax"]
                nc.vector.tensor_tensor_scan(
                    out=pmax, data0=mb, data1=sn, initial=initm,
                    op0=ALU.mult, op1=ALU.max,
                )
                m = pool.tile([P, FL], F32, name="m", tag="m")[:, 0:Fw]
                nc.vector.tensor_tensor_scan(
                    out=rev(m), data0=rev(mbx), data1=rev(pmax), initial=0.0,
                    op0=ALU.mult, op1=ALU.max,
                )
                carry["pmax"] = pmax[:, F_c - 1 : F_c]

                hot = pool.tile([P, FL], U8, name="hot", tag="hot")[:, 0:Fw]
                nc.vector.tensor_tensor(out=hot, in0=sn, in1=m, op=ALU.is_equal)

                if first:
                    nc.gpsimd.tensor_copy(out=smH[:], in_=m[:, 0:look])
                if last:
                    nc.gpsimd.tensor_copy(out=smT[:], in_=m[:, F_c - look : F_c])

                a = look if first else 0
                b = F_c - look if last else F_c
                nc.sync.dma_start(soft_d.ap()[:, c0 + a : c0 + b], soft[:, a:b])
                nc.sync.dma_start(hot_d.ap()[:, c0 + a : c0 + b], hot[:, a:b])

            for it in range(nch + 2):
                if 0 <= it - 1 < nch:
                    sums_cast(it - 1)
                if it < nch:
                    produce(it)
                if 0 <= it - 1 < nch:
                    sums(it - 1)
                if 0 <= it - 2 < nch:
                    maxhot(it - 2)

            # ---------------- cross-partition fixup ----------------
            ones = fx.tile([P, look], F32, tag="ones")
            lm = fx.tile([P, look], F32, tag="lm")
            fm = fx.tile([P, look], F32, tag="fm")
            mbHf = fx.tile([P, look], F32, tag="mbHf")
            mbTf = fx.tile([P, look], F32, tag="mbTf")
            TS = fx.tile([P, 1], F32, tag="TS")
            HS = fx.tile([P, 1], F32, tag="HS")
            TS_sh = fx.tile([P, 1], F32, tag="TS_sh")
            TB = fx.tile([P, 1], F32, tag="TB")
            TBd = fx.tile([P, 1], F32, tag="TBd")
            rB = fx.tile([P, 1], F32, tag="rB")
            rT = fx.tile([P, 1], F32, tag="rT")
            contU = fx.tile([P, 1], F32, tag="contU")
            tmpH = fx.tile([P, look], F32, tag="tmpH")
            tmpT = fx.tile([P, look], F32, tag="tmpT")
            affH = fx.tile([P, look], F32, tag="affH")
            affT = fx.tile([P, look], F32, tag="affT")
            softHn = fx.tile([P, look], F32, tag="softHn")
            softTn = fx.tile([P, look], F32, tag="softTn")
            snHn = fx.tile([P, look], F32, tag="snHn")
            snTn = fx.tile([P, look], F32, tag="snTn")
            mH = fx.tile([P, 1], F32, tag="mH")
            mT = fx.tile([P, 1], F32, tag="mT")
            mTd = fx.tile([P, 1], F32, tag="mTd")
            mHu = fx.tile([P, 1], F32, tag="mHu")
            rmH = fx.tile([P, 1], F32, tag="rmH")
            rmT = fx.tile([P, 1], F32, tag="rmT")
            e1 = fx.tile([P, look], F32, tag="e1")
            e0 = fx.tile([P, look], F32, tag="e0")
            hfH = fx.tile([P, look], F32, tag="hfH")
            hfT = fx.tile([P, look], F32, tag="hfT")
            affHu = fx.tile([P, look], U8, tag="affHu")
            affTu = fx.tile([P, look], U8, tag="affTu")

            # fm[k] = all(mb[1..k]) within head window; fm[0] = 1
            nc.vector.memset(ones[:], 1.0)
            nc.vector.tensor_copy(out=mbHf[:], in_=mbH[:])
            nc.vector.memset(mbHf[:, 0:1], 1.0)
            nc.vector.tensor_tensor_scan(
                out=fm[:], data0=mbHf[:], data1=ones[:], initial=1.0,
                op0=ALU.mult, op1=ALU.mult,
            )
            # lm[k] = all(mbT[k+1..63]) within tail window; lm[63] = 1
            nc.vector.tensor_copy(out=mbTf[:], in_=mbTx[:])
            nc.vector.tensor_tensor_scan(
                out=rev(lm[:]), data0=rev(mbTf[:]), data1=rev(ones[:]), initial=1.0,
                op0=ALU.mult, op1=ALU.mult,
            )
            # cont[0] = 0: first row of a shard always starts a new segment
            nc.vector.memset(cont[0:1, :], 0)

            # tail/head partial sums of e over the boundary run
            nc.vector.tensor_tensor(out=tmpT[:], in0=eT[:], in1=lm[:], op=ALU.mult)
            nc.vector.tensor_reduce(
                out=TS[:], in_=tmpT[:], axis=mybir.AxisListType.X, op=ALU.add
            )
            nc.vector.tensor_tensor(out=tmpH[:], in0=eH[:], in1=fm[:], op=ALU.mult)
            nc.vector.tensor_reduce(
                out=HS[:], in_=tmpH[:], axis=mybir.AxisListType.X, op=ALU.add
            )
            nc.vector.memset(TS_sh[:], 1.0)
            nc.sync.dma_start(TS_sh[1:P, :], TS[0 : P - 1, :])
            nc.vector.tensor_tensor(out=TB[:], in0=TS_sh[:], in1=HS[:], op=ALU.add)
            nc.vector.tensor_scalar(
                out=TB[:], in0=TB[:], scalar1=1e-30, scalar2=None, op0=ALU.max
            )
            nc.vector.memset(TBd[:], 1.0)
            nc.sync.dma_start(TBd[0 : P - 1, :], TB[1:P, :])
            nc.vector.memset(contU[:], 0.0)
            nc.sync.dma_start(contU[0 : P - 1, :], cont[1:P, :])
            nc.vector.reciprocal(rB[:], TB[:])
            nc.vector.reciprocal(rT[:], TBd[:])

            # corrected values, head side
            nc.vector.tensor_scalar(
                out=affH[:], in0=fm[:], scalar1=cont[:], scalar2=None, op0=ALU.mult
            )
            nc.vector.tensor_scalar(
                out=softHn[:], in0=eH[:], scalar1=rB[:], scalar2=None, op0=ALU.mult
            )
            nc.vector.tensor_tensor(out=snHn[:], in0=softHn[:], in1=ueH[:], op=ALU.add)
            # corrected values, tail side
            nc.vector.tensor_scalar(
                out=affT[:], in0=lm[:], scalar1=contU[:], scalar2=None, op0=ALU.mult
            )
            nc.vector.tensor_scalar(
                out=softTn[:], in0=eT[:], scalar1=rT[:], scalar2=None, op0=ALU.mult
            )
            nc.vector.tensor_tensor(out=snTn[:], in0=softTn[:], in1=ueT[:], op=ALU.add)
            # per-side run maxima over affected elements
            nc.vector.tensor_tensor(out=tmpH[:], in0=snHn[:], in1=affH[:], op=ALU.mult)
            nc.vector.tensor_reduce(
                out=mH[:], in_=tmpH[:], axis=mybir.AxisListType.X, op=ALU.max
            )
            nc.vector.tensor_tensor(out=tmpT[:], in0=snTn[:], in1=affT[:], op=ALU.mult)
            nc.vector.tensor_reduce(
                out=mT[:], in_=tmpT[:], axis=mybir.AxisListType.X, op=ALU.max
            )
            nc.vector.memset(mTd[:], 0.0)
            nc.sync.dma_start(mTd[1:P, :], mT[0 : P - 1, :])
            nc.vector.memset(mHu[:], 0.0)
            nc.sync.dma_start(mHu[0 : P - 1, :], mH[1:P, :])
            nc.vector.tensor_tensor(out=rmH[:], in0=mTd[:], in1=mH[:], op=ALU.max)
            nc.vector.tensor_tensor(out=rmT[:], in0=mT[:], in1=mHu[:], op=ALU.max)

            # integer masks for select
            nc.vector.tensor_copy(out=affHu[:], in_=affH[:])
            nc.vector.tensor_copy(out=affTu[:], in_=affT[:])

            # merged soft / hot, head side
            nc.vector.select(softH[:], affHu[:], softHn[:], softH[:])
            nc.vector.tensor_scalar(
                out=e1[:], in0=snHn[:], scalar1=rmH[:], scalar2=None, op0=ALU.is_equal
            )
            nc.vector.tensor_tensor(out=e0[:], in0=snH[:], in1=smH[:], op=ALU.is_equal)
            nc.vector.select(hfH[:], affHu[:], e1[:], e0[:])
            hu8H = fx.tile([P, look], U8, tag="hu8H")
            nc.vector.tensor_copy(out=hu8H[:], in_=hfH[:])
            # merged, tail side
            nc.vector.select(softT[:], affTu[:], softTn[:], softT[:])
            nc.vector.tensor_scalar(
                out=e1[:], in0=snTn[:], scalar1=rmT[:], scalar2=None, op0=ALU.is_equal
            )
            nc.vector.tensor_tensor(out=e0[:], in0=snT[:], in1=smT[:], op=ALU.is_equal)
            nc.vector.select(hfT[:], affTu[:], e1[:], e0[:])
            hu8T = fx.tile([P, look], U8, tag="hu8T")
            nc.vector.tensor_copy(out=hu8T[:], in_=hfT[:])

            nc.sync.dma_start(soft_d.ap()[:, 0:look], softH[:])
            nc.sync.dma_start(hot_d.ap()[:, 0:look], hu8H[:])
            nc.sync.dma_start(soft_d.ap()[:, W - look : W], softT[:])
            nc.sync.dma_start(hot_d.ap()[:, W - look : W], hu8T[:])
    nc.compile()
    return nc


def _prep_shards(logits, logit_groups, u_gumbel, u_eps, W):
    """Split at group boundaries, pad each shard to [P, W] arrays."""
    E = logits.shape[0]
    splits = [0]
    for k in range(1, NCORES):
        t = k * E // NCORES
        splits.append(int(np.searchsorted(logit_groups, logit_groups[t])))
    splits.append(E)

    mb_full = np.zeros(E, dtype=np.float32)
    np.equal(logit_groups[1:], logit_groups[:-1], out=mb_full[1:])

    S = P * W
    in_maps = []
    lens = []
    for k in range(NCORES):
        lo, hi = splits[k], splits[k + 1]
        L = hi - lo
        assert L <= S, (L, S)
        lens.append(L)

        def padded(x, fill, dtype, w=W):
            arr = np.full((P, w), fill, dtype=dtype)
            flat = arr[:, :W].reshape(-1)
            flat[:L] = x
            arr[:, :W] = flat.reshape(P, W)
            return arr

        # row starts keep their true cross-row continuation bit via the
        # [P, W] reshape; padding and the extra column are 0 (every pad
        # element is its own one-element segment). mb[lo] is 0 because
        # shards are snapped to group boundaries.
        mba = padded(mb_full[lo:hi], 0, ml_dtypes.bfloat16, W + 1)
        mba[:, W] = 0
        ua = padded(u_gumbel[lo:hi], 0.5, np.float32)
        la = padded(logits[lo:hi], 0.0, np.float32)
        uea = padded(np.float32(EPS_NOISE) * u_eps[lo:hi], 0.5, np.float32)
        in_maps.append({"u": ua, "l": la, "ue": uea, "mb": mba,
                        "ident": np.eye(P, dtype=ml_dtypes.bfloat16)})
    return in_maps, lens


_CACHE = {}


def kernel(logits, logit_groups, n_groups, u_gumbel, u_eps):
    logits = np.asarray(logits, dtype=np.float32)
    logit_groups = np.asarray(logit_groups, dtype=np.int32)
    u_gumbel = np.asarray(u_gumbel, dtype=np.float32)
    u_eps = np.asarray(u_eps, dtype=np.float32)
    E = logits.shape[0]

    in_maps, lens = _prep_shards(logits, logit_groups, u_gumbel, u_eps, W_FULL)

    if "nc" not in _CACHE:
        _CACHE["nc"] = build(W_FULL, F_FULL)
    nc = _CACHE["nc"]

    res = run_bass_kernel_spmd(nc, in_maps, core_ids=list(range(NCORES)))
    _CACHE["last_res"] = res
    soft = np.empty(E, dtype=np.float32)
    hot = np.empty(E, dtype=np.uint8)
    off = 0
    for k in range(NCORES):
        L = lens[k]
        soft[off : off + L] = res.results[k]["soft"].reshape(-1)[:L]
        hot[off : off + L] = res.results[k]["hot"].reshape(-1)[:L]
        off += L
    assert off == E
    s_hot = hot.astype(np.int32)
    st = hot.astype(np.float32)
    return st, s_hot, soft
